# revision 1
# baseline (speedup 1.0000x reference)
"""Trainium2 Bass kernel for a pre-norm transformer encoder layer.

Problem: x[2,2048,1024]; LN1 -> QKV (16 heads x 64) -> softmax(QK^T) V
-> wo -> +res -> LN2 -> GELU(h@w1+b1)@w2+b2 -> +res.

Sharding: token-parallel over B*N = 4096 tokens; each of the 8 cores owns
512 tokens (cores 0-3: batch 0, cores 4-7: batch 1). Each core recomputes
K/V for its whole batch (no collectives). All activations are kept in
transposed layout [feature, token] so every matmul contracts over the
partition dim. Host pre-rotates each core's batch so its own 512 tokens
are always columns 0:512 -> one NEFF shared by all 8 cores.

Matmuls run in bf16 with fp32 PSUM accumulation; LN stats, softmax,
residuals and GELU run in fp32. Cross-partition reductions (LN stats,
broadcast of per-token scalars) use ones-vector matmuls on the PE.

SBUF pools: persistent tiles on the "right" stack; phase-scoped pools
nest LIFO on the "left" stack.
"""
import sys
sys.path.insert(0, "/opt/trn_rl_repo")

import numpy as np
import ml_dtypes

import concourse.bass as bass
import concourse.tile as tile
from concourse import bacc, mybir

B, N, D = 2, 2048, 1024
H, DH = 16, 64
FF = 4096
NCORES = 8
T = N * B // NCORES          # 512 tokens per core
CPB = NCORES // B            # 4 cores per batch
ET = D // 128                # 8 embed tiles
FT = FF // 128               # 32 ffn tiles
NT = N // 128                # 16 key tiles per batch
NCH = N // 512               # 4 512-chunks per batch
HP = H // 2                  # 8 head pairs

dtb = mybir.dt.bfloat16
dtf = mybir.dt.float32
AF = mybir.ActivationFunctionType
ts = bass.ts


def _ln_chunk(nc, pool, sum_ps, ssq_ps, A_sl, B_sl, eps_ap, chw):
    """From per-token partition sums (sum, sumsq) [1, chw] write fp32
    A = rstd, B = -mean*rstd into the given [1, chw] AP slices."""
    mean = pool.tile([1, chw], dtf, tag="ln_mean")
    var = pool.tile([1, chw], dtf, tag="ln_var")
    sd = pool.tile([1, chw], dtf, tag="ln_sd")
    m2 = pool.tile([1, chw], dtf, tag="ln_m2")
    nc.vector.tensor_scalar_mul(mean[:], sum_ps[:1], 1.0 / D)
    # var = sumsq/D - mean^2
    nc.vector.tensor_scalar_mul(var[:], ssq_ps[:1], 1.0 / D)
    nc.vector.tensor_mul(m2[:], mean[:], mean[:])
    nc.vector.tensor_sub(var[:], var[:], m2[:])
    # A = 1/sqrt(var+eps)
    nc.scalar.activation(sd[:], var[:], AF.Sqrt, bias=eps_ap)
    nc.vector.reciprocal(A_sl, sd[:])
    # B = -mean*rstd
    nc.vector.tensor_mul(m2[:], mean[:], A_sl)
    nc.vector.tensor_scalar_mul(B_sl, m2[:], -1.0)


def build(stage_limit="E2"):
    nc = bacc.Bacc("TRN2", target_bir_lowering=False, debug=False)

    xbT_d = nc.dram_tensor("xbT", [D, N], dtb, kind="ExternalInput").ap()
    xoT_d = nc.dram_tensor("xoT", [D, T], dtf, kind="ExternalInput").ap()
    wq_d = nc.dram_tensor("wq", [D, D], dtb, kind="ExternalInput").ap()
    wk_d = nc.dram_tensor("wk", [D, D], dtb, kind="ExternalInput").ap()
    wv_d = nc.dram_tensor("wv", [D, D], dtb, kind="ExternalInput").ap()
    wo_d = nc.dram_tensor("wo", [D, D], dtb, kind="ExternalInput").ap()
    w1_d = nc.dram_tensor("w1", [D, FF], dtb, kind="ExternalInput").ap()
    w2_d = nc.dram_tensor("w2", [FF, D], dtb, kind="ExternalInput").ap()
    # per-partition vectors, host-prepared as [128, ntiles]
    ln1g_d = nc.dram_tensor("ln1g", [128, ET], dtf, kind="ExternalInput").ap()
    ln1b_d = nc.dram_tensor("ln1b", [128, ET], dtf, kind="ExternalInput").ap()
    ln2g_d = nc.dram_tensor("ln2g", [128, ET], dtf, kind="ExternalInput").ap()
    ln2b_d = nc.dram_tensor("ln2b", [128, ET], dtf, kind="ExternalInput").ap()
    b1_d = nc.dram_tensor("b1", [128, FT], dtf, kind="ExternalInput").ap()
    b2_d = nc.dram_tensor("b2", [128, ET], dtf, kind="ExternalInput").ap()
    outT_d = nc.dram_tensor("outT", [D, T], dtf, kind="ExternalOutput").ap()

    with tile.TileContext(nc) as tc:
        _body(nc, tc, xbT_d, xoT_d, wq_d, wk_d, wv_d, wo_d, w1_d, w2_d,
              ln1g_d, ln1b_d, ln2g_d, ln2b_d, b1_d, b2_d, outT_d,
              stage_limit=stage_limit)
    nc.finalize()
    return nc


def _body(nc, tc, xbT_d, xoT_d, wq_d, wk_d, wv_d, wo_d, w1_d, w2_d,
          ln1g_d, ln1b_d, ln2g_d, ln2b_d, b1_d, b2_d, outT_d,
          stage_limit="E2"):
    mm = nc.tensor.matmul

    def pool(name, bufs, space="SBUF", side=None):
        cm = tc.tile_pool(name=name, bufs=bufs, space=space, side=side)
        return cm, cm.__enter__()

    def close(*cms):
        for cm in cms:
            cm.__exit__(None, None, None)

    # ---------- persistent pools (right stack; closed at the very end) ----
    cpool_cm, cpool = pool("const", 1, side="right")
    x2_cm, x2p = pool("x2", ET, side="right")
    h2_cm, h2p = pool("h2", ET, side="right")
    oall_cm, oallp = pool("oall", HP, side="right")

    ones_bf = cpool.tile([128, 1], dtb)
    nc.vector.memset(ones_bf[:], 1.0)
    ones1_f = cpool.tile([1, 128], dtf)
    nc.vector.memset(ones1_f[:], 1.0)
    eps_t = cpool.tile([1, 1], dtf)
    nc.vector.memset(eps_t[:], 1e-5)
    ln1g = cpool.tile([128, ET], dtf)
    ln1b = cpool.tile([128, ET], dtf)
    ln2g = cpool.tile([128, ET], dtf)
    ln2b = cpool.tile([128, ET], dtf)
    b1s = cpool.tile([128, FT], dtf)
    b2s = cpool.tile([128, ET], dtf)
    for t_, d_ in ((ln1g, ln1g_d), (ln1b, ln1b_d), (ln2g, ln2g_d),
                   (ln2b, ln2b_d), (b1s, b1_d), (b2s, b2_d)):
        nc.sync.dma_start(out=t_[:], in_=d_[:, :])

    # ---------- left stack: pools living through attention ----------
    qt_cm, qtp = pool("qt", HP)
    kt_cm, ktp = pool("kt", HP)
    v_cm, vp = pool("v", NT)

    # ================= stage A: LN1 over the full (rotated) batch =========
    hT_cm, hTp = pool("hT", ET)
    xb_cm, xbp = pool("xb", ET)
    sqa_cm, sqap = pool("sqa", 4)
    lns_cm, lnsp = pool("lns", 1)
    sps_cm, spsp = pool("sps", NCH, space="PSUM")
    ssqs_cm, ssqp = pool("ssqs", NCH, space="PSUM")

    A1 = lnsp.tile([1, N], dtf, tag="ln_A")
    B1v = lnsp.tile([1, N], dtf, tag="ln_B")
    sum_ps = [spsp.tile([1, 512], dtf, tag="sum", name=f"sum{_c}") for _c in range(NCH)]
    ssq_ps = [ssqp.tile([1, 512], dtf, tag="ssq", name=f"ssq{_c}") for _c in range(NCH)]
    xb = []
    for e in range(ET):
        xt = xbp.tile([128, N], dtb, tag="xb")
        nc.sync.dma_start(out=xt[:], in_=xbT_d[ts(e, 128), :])
        xb.append(xt)
        for c in range(NCH):
            sq = sqap.tile([128, 512], dtb, tag="sq")
            nc.vector.tensor_mul(sq[:], xt[:, ts(c, 512)], xt[:, ts(c, 512)])
            mm(sum_ps[c][:1], ones_bf[:], xt[:, ts(c, 512)],
               start=(e == 0), stop=(e == ET - 1))
            mm(ssq_ps[c][:1], ones_bf[:], sq[:],
               start=(e == 0), stop=(e == ET - 1))
    for c in range(NCH):
        _ln_chunk(nc, lnsp, sum_ps[c][:], ssq_ps[c][:],
                  A1[:, ts(c, 512)], B1v[:, ts(c, 512)], eps_t[:], 512)
    close(ssqs_cm, sps_cm)

    ua_cm, uap = pool("ua", 3)
    bca_cm, bcap = pool("bca", 2, space="PSUM")
    hT = [hTp.tile([128, N], dtb, tag="hT", name=f"hT{_e}") for _e in range(ET)]
    for c in range(NCH):
        Abc = bcap.tile([128, 512], dtf, tag="Abc")
        Bbc = bcap.tile([128, 512], dtf, tag="Bbc")
        mm(Abc[:], ones1_f[:], A1[:, ts(c, 512)], start=True, stop=True)
        mm(Bbc[:], ones1_f[:], B1v[:, ts(c, 512)], start=True, stop=True)
        # bf16 SBUF copies so the normalize ops hit the DVE fast modes
        Acb = uap.tile([128, 512], dtb, tag="Acb")
        Bcb = uap.tile([128, 512], dtb, tag="Bcb")
        nc.vector.tensor_copy(Acb[:], Abc[:])
        nc.vector.tensor_copy(Bcb[:], Bbc[:])
        for e in range(ET):
            u = uap.tile([128, 512], dtb, tag="u")
            nc.vector.tensor_mul(u[:], xb[e][:, ts(c, 512)], Acb[:])
            nc.vector.tensor_add(u[:], u[:], Bcb[:])
            nc.vector.tensor_scalar(
                out=hT[e][:, ts(c, 512)], in0=u[:],
                scalar1=ln1g[:, e:e + 1], scalar2=ln1b[:, e:e + 1],
                op0=mybir.AluOpType.mult, op1=mybir.AluOpType.add)
    close(bca_cm, ua_cm, lns_cm, sqa_cm, xb_cm)
    if stage_limit == "A":
        close(hT_cm, v_cm, kt_cm, qt_cm, oall_cm, h2_cm, x2_cm, cpool_cm)
        return

    # ================= stage B: Q,K (transposed, head pairs) and V ========
    wq_cm, wqp = pool("wq", ET)
    qps_cm, qpsp = pool("qps", 3, space="PSUM")
    wq_sb = []
    for e in range(ET):
        tq = wqp.tile([128, D], dtb, tag="wq")
        nc.sync.dma_start(out=tq[:], in_=wq_d[ts(e, 128), :])
        wq_sb.append(tq)
    qt = []
    for p in range(HP):
        q_ps = qpsp.tile([128, 512], dtf, tag="qps")
        for e in range(ET):
            mm(q_ps[:], wq_sb[e][:, ts(p, 128)], hT[e][:, 0:T],
               start=(e == 0), stop=(e == ET - 1))
        tq = qtp.tile([128, T], dtb, tag="qt")
        nc.vector.tensor_copy(tq[:], q_ps[:])
        qt.append(tq)
    close(qps_cm, wq_cm)

    wk_cm, wkp = pool("wk", ET)
    kps_cm, kpsp = pool("kps", 4, space="PSUM")
    wk_sb = []
    for e in range(ET):
        tk = wkp.tile([128, D], dtb, tag="wk")
        nc.sync.dma_start(out=tk[:], in_=wk_d[ts(e, 128), :])
        wk_sb.append(tk)
    kt = []
    for p in range(HP):
        tk = ktp.tile([128, N], dtb, tag="kt")
        for c in range(NCH):
            k_ps = kpsp.tile([128, 512], dtf, tag="kps")
            for e in range(ET):
                mm(k_ps[:], wk_sb[e][:, ts(p, 128)], hT[e][:, ts(c, 512)],
                   start=(e == 0), stop=(e == ET - 1))
            nc.vector.tensor_copy(tk[:, ts(c, 512)], k_ps[:])
        kt.append(tk)
    close(kps_cm, wk_cm)

    wv_cm, wvp = pool("wv", ET)
    vps_cm, vpsp = pool("vps", 4, space="PSUM")
    wv_sb = []
    for e in range(ET):
        tv = wvp.tile([128, D], dtb, tag="wv")
        nc.sync.dma_start(out=tv[:], in_=wv_d[ts(e, 128), :])
        wv_sb.append(tv)
    v_sb = []
    for t in range(NT):
        vt = vp.tile([128, H * (DH + 1)], dtb, tag="v")
        v3 = vt[:].rearrange("p (h c) -> p h c", c=DH + 1)
        nc.vector.memset(v3[:, :, DH:DH + 1], 1.0)
        for c2 in range(2):
            v_ps = vpsp.tile([128, 512], dtf, tag="vps")
            for e in range(ET):
                mm(v_ps[:], hT[e][:, ts(t, 128)], wv_sb[e][:, ts(c2, 512)],
                   start=(e == 0), stop=(e == ET - 1))
            nc.vector.tensor_copy(
                v3[:, c2 * 8:(c2 + 1) * 8, 0:DH],
                v_ps[:].rearrange("p (h c) -> p h c", c=DH))
        v_sb.append(vt)
    close(vps_cm, wv_cm, hT_cm)
    if stage_limit == "B":
        close(v_cm, kt_cm, qt_cm, oall_cm, h2_cm, x2_cm, cpool_cm)
        return

    # ================= stage C: attention =================
    # wo + residual prefetch (right stack; popped right after stage D)
    xo_cm, xop = pool("xo", ET, side="right")
    wo_cm, wop = pool("wo", ET, side="right")
    wo_sb, xo_sb = [], []
    for e in range(ET):
        tw = wop.tile([128, D], dtb, tag="wo")
        nc.sync.dma_start(out=tw[:], in_=wo_d[ts(e, 128), :])
        wo_sb.append(tw)
        tx = xop.tile([128, T], dtf, tag="xo")
        nc.sync.dma_start(out=tx[:], in_=xoT_d[ts(e, 128), :])
        xo_sb.append(tx)

    pt_cm, ptp = pool("pt", 8)
    osb_cm, osbp = pool("osb", 4)
    rec_cm, recp = pool("rec", 4)
    dps_cm, dpsp = pool("dps", 3, space="PSUM")
    ops_cm, opsp = pool("ops", 1, space="PSUM")
    rbs_cm, rbsp = pool("rbs", 1, space="PSUM")

    oall = [oallp.tile([128, T], dtb, tag="oall", name=f"oall{_p}") for _p in range(HP)]

    for p in range(HP):
        hA, hB = 2 * p, 2 * p + 1
        oA_ps = opsp.tile([DH + 1, T], dtf, tag="opsA")
        oB_ps = opsp.tile([DH + 1, T], dtf, tag="opsB")
        for j in range(NT):
            # both heads' dots in one 2-bank tile; row-groups (0,0)/(64,0)
            # run concurrently in the PE array, and one Exp covers both.
            dp_ps = dpsp.tile([128, 2 * T], dtf, tag="dps")
            mm(dp_ps[:, 0:T], kt[p][0:64, ts(j, 128)], qt[p][0:64, :],
               start=True, stop=True)
            mm(dp_ps[:, T:2 * T], kt[p][64:128, ts(j, 128)], qt[p][64:128, :],
               start=True, stop=True)
            pt = ptp.tile([128, 2 * T], dtb, tag="pt")
            nc.scalar.activation(pt[:], dp_ps[:], AF.Exp)
            mm(oA_ps[:], v_sb[j][:, hA * (DH + 1):(hA + 1) * (DH + 1)],
               pt[:, 0:T], start=(j == 0), stop=(j == NT - 1))
            mm(oB_ps[:], v_sb[j][:, hB * (DH + 1):(hB + 1) * (DH + 1)],
               pt[:, T:2 * T], start=(j == 0), stop=(j == NT - 1))
        rb_ps = dpsp.tile([64, 2 * T], dtf, tag="dps", name=f"rb{p}")
        for o_ps, off, rsl in ((oA_ps, 0, slice(0, T)), (oB_ps, 64, slice(T, 2 * T))):
            rec = recp.tile([1, T], dtf, tag="rec")
            nc.vector.reciprocal(rec[:], o_ps[DH:DH + 1, :])
            mm(rb_ps[:, rsl], ones1_f[:, 0:64], rec[:], start=True, stop=True)
            osb = osbp.tile([64, T], dtf, tag="osb")
            nc.vector.tensor_copy(osb[:], o_ps[0:DH, :])
            nc.vector.tensor_mul(oall[p][off:off + 64, :], osb[:], rb_ps[:, rsl])
    close(rbs_cm, ops_cm, dps_cm, rec_cm, osb_cm, pt_cm)
    close(v_cm, kt_cm, qt_cm)
    if stage_limit == "C":
        close(wo_cm, xo_cm, oall_cm, h2_cm, x2_cm, cpool_cm)
        return

    # FFN weight pools opened early (LIFO: close after stage D pools);
    # their DMAs overlap stage D.
    g_cm, gp = pool("g", FT)
    w2_cm, w2p = pool("w2", 8)
    w1_cm, w1p = pool("w1", ET)
    w1_sb = []
    for e in range(ET):
        tw1 = w1p.tile([128, FF], dtb, tag="w1")
        nc.sync.dma_start(out=tw1[:], in_=w1_d[ts(e, 128), :])
        w1_sb.append(tw1)
    w2_sb = []
    for f in range(FT):
        tw2 = w2p.tile([128, D], dtb, tag="w2", name=f"w2_{f}")
        nc.sync.dma_start(out=tw2[:], in_=w2_d[ts(f, 128), :])
        w2_sb.append(tw2)

    # ================= stage D: wo proj + residual + LN2 =================
    x2b_cm, x2bp = pool("x2b", ET)
    sqd_cm, sqdp = pool("sqd", 2)
    und_cm, undp = pool("und", 3)
    lns2_cm, lns2p = pool("lns2", 1)
    prs_cm, prsp = pool("prs", 3, space="PSUM")
    s2s_cm, s2sp = pool("s2s", 1, space="PSUM")
    ss2s_cm, ss2p = pool("ss2s", 1, space="PSUM")
    bcd_cm, bcdp = pool("bcd", 1, space="PSUM")

    x2, x2b = [], []
    s2_ps = s2sp.tile([1, T], dtf, tag="s2")
    ss2_ps = ss2p.tile([1, T], dtf, tag="ss2")
    for e in range(ET):
        pr_ps = prsp.tile([128, T], dtf, tag="prs")
        for c in range(ET):
            mm(pr_ps[:], wo_sb[c][:, ts(e, 128)], oall[c][:],
               start=(c == 0), stop=(c == ET - 1))
        tx2 = x2p.tile([128, T], dtf, tag="x2")
        nc.vector.tensor_add(tx2[:], pr_ps[:], xo_sb[e][:])
        x2.append(tx2)
        tb = x2bp.tile([128, T], dtb, tag="x2b")
        nc.vector.tensor_copy(tb[:], tx2[:])
        x2b.append(tb)
        sq = sqdp.tile([128, T], dtb, tag="sqd")
        nc.vector.tensor_mul(sq[:], tb[:], tb[:])
        mm(s2_ps[:1], ones_bf[:], tb[:], start=(e == 0), stop=(e == ET - 1))
        mm(ss2_ps[:1], ones_bf[:], sq[:], start=(e == 0), stop=(e == ET - 1))

    A2 = lns2p.tile([1, T], dtf, tag="ln_A2")
    B2v = lns2p.tile([1, T], dtf, tag="ln_B2")
    _ln_chunk(nc, lns2p, s2_ps[:], ss2_ps[:], A2[:], B2v[:], eps_t[:], T)
    A2bc = bcdp.tile([128, T], dtf, tag="A2bc")
    B2bc = bcdp.tile([128, T], dtf, tag="B2bc")
    mm(A2bc[:], ones1_f[:], A2[:], start=True, stop=True)
    mm(B2bc[:], ones1_f[:], B2v[:], start=True, stop=True)
    A2cb = undp.tile([128, T], dtb, tag="A2cb")
    B2cb = undp.tile([128, T], dtb, tag="B2cb")
    nc.vector.tensor_copy(A2cb[:], A2bc[:])
    nc.vector.tensor_copy(B2cb[:], B2bc[:])
    h2 = []
    for e in range(ET):
        u = undp.tile([128, T], dtb, tag="und")
        nc.vector.tensor_mul(u[:], x2b[e][:], A2cb[:])
        nc.vector.tensor_add(u[:], u[:], B2cb[:])
        th = h2p.tile([128, T], dtb, tag="h2")
        nc.vector.tensor_scalar(
            out=th[:], in0=u[:],
            scalar1=ln2g[:, e:e + 1], scalar2=ln2b[:, e:e + 1],
            op0=mybir.AluOpType.mult, op1=mybir.AluOpType.add)
        h2.append(th)
    close(bcd_cm, ss2s_cm, s2s_cm, prs_cm, lns2_cm, und_cm, sqd_cm, x2b_cm)
    close(wo_cm, xo_cm, oall_cm)
    if stage_limit == "D":
        close(w1_cm, w2_cm, g_cm, h2_cm, x2_cm, cpool_cm)
        return

    # ================= stage E: FFN =================
    aps_cm, apsp = pool("aps", 3, space="PSUM")
    g_sb = []
    for f in range(FT):
        a_ps = apsp.tile([128, T], dtf, tag="aps")
        for e in range(ET):
            mm(a_ps[:], w1_sb[e][:, ts(f, 128)], h2[e][:],
               start=(e == 0), stop=(e == ET - 1))
        tg = gp.tile([128, T], dtb, tag="g")
        nc.scalar.activation(tg[:], a_ps[:], AF.Gelu, bias=b1s[:, f:f + 1])
        g_sb.append(tg)
    close(aps_cm)
    if stage_limit == "E1":
        close(w1_cm, w2_cm, g_cm, h2_cm, x2_cm, cpool_cm)
        return

    ob_cm, obp = pool("ob", 4)
    yps_cm, ypsp = pool("yps", ET, space="PSUM")
    y_ps = [ypsp.tile([128, T], dtf, tag="yps", name=f"yps{_e}")
            for _e in range(ET)]
    for f in range(FT):
        for e in range(ET):
            mm(y_ps[e][:], w2_sb[f][:, ts(e, 128)], g_sb[f][:],
               start=(f == 0), stop=(f == FT - 1))
    for e in range(ET):
        ob = obp.tile([128, T], dtf, tag="ob")
        nc.vector.tensor_add(ob[:], y_ps[e][:], x2[e][:])
        nc.vector.tensor_scalar_add(ob[:], ob[:], b2s[:, e:e + 1])
        nc.sync.dma_start(out=outT_d[ts(e, 128), :], in_=ob[:])
    close(yps_cm, ob_cm, w1_cm, w2_cm, g_cm)

    # ---------- close persistent (right) pools, reverse order ----------
    close(h2_cm, x2_cm, cpool_cm)


_NC_CACHE = {}


def _get_nc():
    if "nc" not in _NC_CACHE:
        _NC_CACHE["nc"] = build()
    return _NC_CACHE["nc"]


def _vec_tiles(v, ntiles):
    return np.ascontiguousarray(
        np.asarray(v, np.float32).reshape(ntiles, 128).T)


def prepare_in_maps(x, wq, wk, wv, wo, w1, b1, w2, b2,
                    ln1_g, ln1_b, ln2_g, ln2_b):
    bf = ml_dtypes.bfloat16
    x = np.asarray(x, np.float32)
    shared = {
        "wq": np.ascontiguousarray(np.asarray(wq).astype(bf)),
        "wk": np.ascontiguousarray(np.asarray(wk).astype(bf)),
        "wv": np.ascontiguousarray(np.asarray(wv).astype(bf)),
        "wo": np.ascontiguousarray(np.asarray(wo).astype(bf)),
        "w1": np.ascontiguousarray(np.asarray(w1).astype(bf)),
        "w2": np.ascontiguousarray(np.asarray(w2).astype(bf)),
        "ln1g": _vec_tiles(ln1_g, ET), "ln1b": _vec_tiles(ln1_b, ET),
        "ln2g": _vec_tiles(ln2_g, ET), "ln2b": _vec_tiles(ln2_b, ET),
        "b1": _vec_tiles(b1, FT), "b2": _vec_tiles(b2, ET),
    }
    in_maps = []
    for c in range(NCORES):
        b, s = divmod(c, CPB)
        rot = np.concatenate([x[b, s * T:], x[b, :s * T]], axis=0)  # own first
        m = dict(shared)
        m["xbT"] = np.ascontiguousarray(rot.T.astype(bf))
        m["xoT"] = np.ascontiguousarray(x[b, s * T:(s + 1) * T].T)
        in_maps.append(m)
    return in_maps


def assemble_output(results):
    out = np.empty((B, N, D), np.float32)
    for c in range(NCORES):
        b, s = divmod(c, CPB)
        out[b, s * T:(s + 1) * T, :] = results[c]["outT"].T
    return out


def kernel(x, wq, wk, wv, wo, w1, b1, w2, b2, ln1_g, ln1_b, ln2_g, ln2_b):
    from concourse.bass_utils import run_bass_kernel_spmd

    nc = _get_nc()
    in_maps = prepare_in_maps(x, wq, wk, wv, wo, w1, b1, w2, b2,
                              ln1_g, ln1_b, ln2_g, ln2_b)
    res = run_bass_kernel_spmd(nc, in_maps, core_ids=list(range(NCORES)))
    return assemble_output(res.results)



# revision 30
# speedup vs baseline: 1.2637x; 1.2637x over previous
"""Trainium2 Bass kernel for a pre-norm transformer encoder layer.

Problem: x[2,2048,1024]; LN1 -> QKV (16 heads x 64) -> softmax(QK^T) V
-> wo -> +res -> LN2 -> GELU(h@w1+b1)@w2+b2 -> +res.

Sharding: token-parallel over B*N = 4096 tokens; each of the 8 cores owns
512 tokens (cores 0-3: batch 0, cores 4-7: batch 1). Each core recomputes
K/V for its whole batch (no collectives). All activations are kept in
transposed layout [feature, token] so every matmul contracts over the
partition dim. Host pre-rotates each core's batch so its own 512 tokens
are always columns 0:512 -> one NEFF shared by all 8 cores.

v2 restructure (PE-minimizing, cost-model-aware):
 - LN gammas are folded into wq/wk/wv/w1 rows on the host; LN betas become
   per-output-column constants (b@W) applied for free in the PSUM->SBUF
   copies. Device LN is the pure normalize (x-mu)*rstd = x*A - B.
 - LN statistics: sum(x) via a single all-ones [128,128] stationary matmul
   chain (output replicated across partitions); sum(x^2) via a GpSimd
   (Pool-engine) add tree + partition_all_reduce. No 1-row matmuls, no
   broadcast matmuls.
 - Attention AV uses pt (exp dots, keys on partitions) as the *stationary*
   operand so the output is [128 queries, 65] -- full partition use, free
   size 65 instead of 512 (2x less PE time); softmax normalize becomes a
   per-partition tensor_scalar; a PE transpose restores [dh, token] layout.
 - K for head-pair p+1 is computed interleaved inside pair p's attention
   j-loop so the PE fills the gaps left by the (ACT-bound) exp.
 - FFN2 runs e-outer so output tiles drain early (short tail).

Matmuls run in bf16 with fp32 PSUM accumulation.
"""
import sys
sys.path.insert(0, "/opt/trn_rl_repo")

import numpy as np
import ml_dtypes

import concourse.bass as bass
import concourse.bass_isa as bass_isa
import concourse.tile as tile
from concourse import bacc, mybir

B, N, D = 2, 2048, 1024
H, DH = 16, 64
FF = 4096
NCORES = 8
T = N * B // NCORES          # 512 tokens per core
CPB = NCORES // B            # 4 cores per batch
ET = D // 128                # 8 embed tiles
FT = FF // 128               # 32 ffn tiles
NT = N // 128                # 16 key tiles per batch
NCH = N // 512               # 4 512-chunks per batch
HP = H // 2                  # 8 head pairs

dtb = mybir.dt.bfloat16
dtf = mybir.dt.float32
AF = mybir.ActivationFunctionType
RED = bass_isa.ReduceOp
ts = bass.ts


def build(stage_limit="E"):
    nc = bacc.Bacc("TRN2", target_bir_lowering=False, debug=False)

    xbT_d = nc.dram_tensor("xbT", [D, N], dtb, kind="ExternalInput").ap()
    xoT_d = nc.dram_tensor("xoT", [D, T], dtf, kind="ExternalInput").ap()
    wq_d = nc.dram_tensor("wq", [D, D], dtb, kind="ExternalInput").ap()
    wk_d = nc.dram_tensor("wk", [D, D], dtb, kind="ExternalInput").ap()
    wv_d = nc.dram_tensor("wv", [D, D], dtb, kind="ExternalInput").ap()
    wo_d = nc.dram_tensor("wo", [D, D], dtb, kind="ExternalInput").ap()
    w1_d = nc.dram_tensor("w1", [D, FF], dtb, kind="ExternalInput").ap()
    w2_d = nc.dram_tensor("w2", [FF, D], dtb, kind="ExternalInput").ap()
    bq_d = nc.dram_tensor("bq", [128, HP], dtf, kind="ExternalInput").ap()
    bk_d = nc.dram_tensor("bk", [128, HP], dtf, kind="ExternalInput").ap()
    bvw_d = nc.dram_tensor("bvw", [128, D], dtb, kind="ExternalInput").ap()
    b1e_d = nc.dram_tensor("b1e", [128, FT], dtf, kind="ExternalInput").ap()
    b2_d = nc.dram_tensor("b2", [128, ET], dtf, kind="ExternalInput").ap()
    id_d = nc.dram_tensor("ident", [128, 128], dtf, kind="ExternalInput").ap()
    outT_d = nc.dram_tensor("outT", [D, T], dtf, kind="ExternalOutput").ap()

    with tile.TileContext(nc) as tc:
        _body(nc, tc, xbT_d, xoT_d, wq_d, wk_d, wv_d, wo_d, w1_d, w2_d,
              bq_d, bk_d, bvw_d, b1e_d, b2_d, id_d, outT_d, stage_limit)
    nc.finalize()
    return nc


def _body(nc, tc, xbT_d, xoT_d, wq_d, wk_d, wv_d, wo_d, w1_d, w2_d,
          bq_d, bk_d, bvw_d, b1e_d, b2_d, id_d, outT_d, stage_limit):
    mm = nc.tensor.matmul

    def pool(name, bufs, space="SBUF", side=None):
        cm = tc.tile_pool(name=name, bufs=bufs, space=space, side=side)
        return cm, cm.__enter__()

    def close(*cms):
        for cm in cms:
            cm.__exit__(None, None, None)

    # ---------- persistent pools (right stack) ----------
    cpool_cm, cpool = pool("const", 1, side="right")
    x2_cm, x2p = pool("x2", ET, side="right")
    h2_cm, h2p = pool("h2", ET, side="right")
    oall_cm, oallp = pool("oall", HP, side="right")

    ones128 = cpool.tile([128, 128], dtb)
    nc.vector.memset(ones128[:], 1.0)
    eps128 = cpool.tile([128, 1], dtf)
    nc.vector.memset(eps128[:], 1e-5)
    ident = cpool.tile([128, 128], dtf)
    bq_s = cpool.tile([128, HP], dtf)
    bk_s = cpool.tile([128, HP], dtf)
    bvw_s = cpool.tile([128, D], dtb)
    b1e_s = cpool.tile([128, FT], dtf)
    b2_s = cpool.tile([128, ET], dtf)
    for t_, d_ in ((ident, id_d), (bq_s, bq_d), (bk_s, bk_d),
                   (bvw_s, bvw_d), (b1e_s, b1e_d), (b2_s, b2_d)):
        nc.sync.dma_start(out=t_[:], in_=d_[:, :])

    # ---------- left stack: pools living into the attention phase ----------
    hT_cm, hTp = pool("hT", ET)
    v_cm, vp = pool("v", NT)
    qt_cm, qtp = pool("qt", HP)
    wk_cm, wkp = pool("wk", ET)
    kt_cm, ktp = pool("kt", 3)

    # LN-phase pools
    wq_cm, wqp = pool("wq", ET)
    wv_cm, wvp = pool("wv", ET)
    xb_cm, xbp = pool("xb", 2 * ET)
    sq_cm, sqp = pool("sq", ET + 2)
    tr_cm, trp = pool("tr", 1)
    ab_cm, abp = pool("ab", 1)
    sps_cm, spsp = pool("sps", 2, space="PSUM")
    sqps_cm, sqpsp = pool("sqps", 1, space="PSUM")
    qps_cm, qpsp = pool("qps", 2, space="PSUM")
    vps_cm, vpsp = pool("vps", 2, space="PSUM")
    kps0_cm, kps0p = pool("kps0", 1, space="PSUM")

    # DMA order = emission order (single queue): x chunk 0 first, then wq/wv
    # (needed early), then the rest of x, then wk (needed at attention).
    def load_xchunk(c):
        out = []
        for e in range(ET):
            t = xbp.tile([128, 512], dtb, tag="xb", name=f"xb{c}_{e}")
            nc.sync.dma_start(out=t[:], in_=xbT_d[ts(e, 128), ts(c, 512)])
            out.append(t)
        return out

    xbc = {0: load_xchunk(0)}
    wq_sb, wv_sb = [], []
    for e in range(ET):
        tq = wqp.tile([128, D], dtb, tag="wq")
        nc.sync.dma_start(out=tq[:], in_=wq_d[ts(e, 128), :])
        wq_sb.append(tq)
    for e in range(ET):
        tv = wvp.tile([128, D], dtb, tag="wv")
        nc.sync.dma_start(out=tv[:], in_=wv_d[ts(e, 128), :])
        wv_sb.append(tv)
    for c in range(1, NCH):
        xbc[c] = load_xchunk(c)
    wk_sb = []
    for e in range(ET):
        tk = wkp.tile([128, D], dtb, tag="wk")
        nc.sync.dma_start(out=tk[:], in_=wk_d[ts(e, 128), :])
        wk_sb.append(tk)

    hT = [hTp.tile([128, N], dtb, tag="hT", name=f"hT{e}") for e in range(ET)]
    qt = [qtp.tile([128, T], dtb, tag="qt", name=f"qt{p}") for p in range(HP)]
    v_sb = [vp.tile([128, H * (DH + 1)], dtb, tag="v", name=f"v{j}")
            for j in range(NT)]
    kt_tiles = {}

    def alloc_kt(p):
        kt_tiles[p] = ktp.tile([128, N], dtb, tag="kt", name=f"kt{p}")

    alloc_kt(0)
    alloc_kt(1)

    # ============ stage A+B: per-chunk LN1 -> Q(c0) / V(c) / K0(c) ========
    def ln_normalize(pl, x_slices, S_ps, SQr, out_slices, cw):
        """A = rsqrt(var+eps), B = mean*A; out = x*A - B (all [128, cw])."""
        mean = pl.tile([128, cw], dtf, tag="ab_mean")
        var = pl.tile([128, cw], dtf, tag="ab_var")
        m2 = pl.tile([128, cw], dtf, tag="ab_m2")
        Ar = pl.tile([128, cw], dtf, tag="ab_A")
        Acb = pl.tile([128, cw], dtb, tag="ab_Acb")
        Bcb = pl.tile([128, cw], dtb, tag="ab_Bcb")
        nc.vector.tensor_scalar_mul(mean[:], S_ps[:], 1.0 / D)
        nc.vector.tensor_scalar_mul(var[:], SQr[:], 1.0 / D)
        nc.vector.tensor_mul(m2[:], mean[:], mean[:])
        nc.vector.tensor_sub(var[:], var[:], m2[:])
        nc.scalar.activation(var[:], var[:], AF.Sqrt, bias=eps128[:])
        nc.vector.reciprocal(Ar[:], var[:])
        nc.vector.tensor_mul(m2[:], mean[:], Ar[:])   # B = mean*A
        nc.vector.tensor_copy(Acb[:], Ar[:])
        nc.vector.tensor_copy(Bcb[:], m2[:])
        for xsl, osl in zip(x_slices, out_slices):
            nc.vector.tensor_mul(osl, xsl, Acb[:])
            nc.vector.tensor_sub(osl, osl, Bcb[:])

    def squares(sql, x_slices, cw):
        sq = []
        for xsl in x_slices:
            t = sql.tile([128, cw], dtb, tag="sq")
            nc.vector.tensor_mul(t[:], xsl, xsl)
            sq.append(t)
        return sq

    def pool_sumsq(pl, sq, cw):
        """Serial Pool (GpSimd) adds + partition_all_reduce -> [128, cw]."""
        acc = pl.tile([128, cw], dtf, tag="tr_acc")
        nc.gpsimd.tensor_add(acc[:], sq[0][:], sq[1][:])
        for e in range(2, len(sq)):
            nc.gpsimd.tensor_add(acc[:], acc[:], sq[e][:])
        sqr = pl.tile([128, cw], dtf, tag="tr_r")
        nc.gpsimd.partition_all_reduce(sqr[:], acc[:], 128, RED.add)
        return sqr

    def pe_sum(psp, tag, slices, cw):
        """sum over tiles via all-ones stationary matmul chain (replicated)."""
        s = psp.tile([128, cw], dtf, tag=tag)
        for i, sl in enumerate(slices):
            mm(s[:], ones128[:], sl, start=(i == 0), stop=(i == len(slices) - 1))
        return s

    def k_chain(kt_t, p, c, ps_pool, ps_tag):
        k_ps = ps_pool.tile([128, 512], dtf, tag=ps_tag)
        for e in range(ET):
            mm(k_ps[:], wk_sb[e][:, ts(p, 128)], hT[e][:, ts(c, 512)],
               start=(e == 0), stop=(e == ET - 1))
        nc.vector.tensor_scalar_add(kt_t[:, ts(c, 512)], k_ps[:],
                                    bk_s[:, p:p + 1])

    for c in range(NCH):
        csl = ts(c, 512)
        xc = [xbc[c][e][:, :] for e in range(ET)]
        S_ps = pe_sum(spsp, "S", xc, 512)
        sq = squares(sqp, xc, 512)
        if c == 0:
            # chunk 0 is the critical path: sumsq on the (idle) PE instead of
            # the serial Pool tree to cut first-chunk latency
            SQr = pe_sum(sqpsp, "SQ", [t[:] for t in sq], 512)
        else:
            SQr = pool_sumsq(trp, sq, 512)
        ln_normalize(abp, xc, S_ps, SQr,
                     [hT[e][:, csl] for e in range(ET)], 512)
        # Q (own tokens = chunk 0 only)
        if c == 0:
            for p in range(HP):
                q_ps = qpsp.tile([128, T], dtf, tag="qps")
                for e in range(ET):
                    mm(q_ps[:], wq_sb[e][:, ts(p, 128)], hT[e][:, 0:T],
                       start=(e == 0), stop=(e == ET - 1))
                nc.vector.tensor_scalar_add(qt[p][:], q_ps[:], bq_s[:, p:p + 1])
        # V for this chunk's 4 key tiles
        for j in range(4 * c, 4 * c + 4):
            vt = v_sb[j]
            v3 = vt[:].rearrange("p (h c) -> p h c", c=DH + 1)
            nc.vector.memset(v3[:, :, DH:DH + 1], 1.0)
            for c2 in range(2):
                v_ps = vpsp.tile([128, 512], dtf, tag="vps")
                for e in range(ET):
                    mm(v_ps[:], hT[e][:, ts(j, 128)], wv_sb[e][:, ts(c2, 512)],
                       start=(e == 0), stop=(e == ET - 1))
                bsl = bvw_s[:, ts(c2, 512)].rearrange("p (h c) -> p h c", c=DH)
                nc.vector.tensor_add(
                    v3[:, c2 * 8:(c2 + 1) * 8, 0:DH],
                    v_ps[:].rearrange("p (h c) -> p h c", c=DH), bsl)
        # K pair 0 for this chunk (pairs >= 1 fill attention-phase PE gaps)
        k_chain(kt_tiles[0], 0, c, kps0p, "kps0")
    for c in range(NCH):
        k_chain(kt_tiles[1], 1, c, kps0p, "kps0")

    close(kps0_cm, vps_cm, qps_cm, sqps_cm, sps_cm, ab_cm, tr_cm, sq_cm,
          xb_cm, wv_cm, wq_cm)
    if stage_limit == "A":
        close(kt_cm, wk_cm, qt_cm, v_cm, hT_cm,
              oall_cm, h2_cm, x2_cm, cpool_cm)
        return

    # wo + residual prefetch (right stack; popped after stage D)
    xo_cm, xop = pool("xo", ET, side="right")
    wo_cm, wop = pool("wo", ET, side="right")
    wo_sb, xo_sb = [], []
    for e in range(ET):
        tw = wop.tile([128, D], dtb, tag="wo")
        nc.sync.dma_start(out=tw[:], in_=wo_d[ts(e, 128), :])
        wo_sb.append(tw)
        tx = xop.tile([128, T], dtf, tag="xo")
        nc.sync.dma_start(out=tx[:], in_=xoT_d[ts(e, 128), :])
        xo_sb.append(tx)

    # ============ stage C: attention, software-pipelined ============
    # Per pair p's j-loop: dots/exp lead, AV lags LAG slots (so pair p-1's
    # oT drains before AV(p,0) needs its PSUM slot), K chains for pair p+2
    # fill PE gaps, and pair p-1's transposes ride the first 8 slots.
    LAG = 4
    pt_cm, ptp = pool("pt", LAG + 2)
    onr_cm, onrp = pool("onr", 12)
    rec_cm, recp = pool("rec", 4)
    dps_cm, dpsp = pool("dps", 2, space="PSUM")
    ops_cm, opsp = pool("ops", 1, space="PSUM")
    kps_cm, kpsp = pool("kps", 1, space="PSUM")
    trp_cm, trpp = pool("trp", 1, space="PSUM")

    oall = [oallp.tile([128, T], dtb, tag="oall", name=f"oall{p}")
            for p in range(HP)]

    def emit_av(oT, p, j, pt):
        for h2 in range(2):
            voff = (2 * p + h2) * (DH + 1)
            # one accumulation group per 2KB zero region (bank): start
            # zeroes the whole bank, so the 4 qc-chains share one group
            for qc in range(4):
                mm(oT[:, h2 * 512 + qc * 65: h2 * 512 + qc * 65 + 65],
                   pt[:, h2 * T + qc * 128: h2 * T + (qc + 1) * 128],
                   v_sb[j][:, voff: voff + DH + 1],
                   start=(j == 0 and qc == 0),
                   stop=(j == NT - 1 and qc == 3))

    def emit_norm(oT, p):
        """reciprocal + scale for the 8 (head, qchunk) outputs of pair p;
        returns the normalized [128, DH] tiles (transposed next pair)."""
        out = []
        for i in range(8):
            h2, qc = divmod(i, 4)
            base = h2 * 512 + qc * 65
            rec = recp.tile([128, 1], dtf, tag="rec")
            nc.vector.reciprocal(rec[:], oT[:, base + DH: base + DH + 1])
            onr = onrp.tile([128, DH], dtf, tag="onr", name=f"onr{p}_{i}")
            nc.vector.tensor_scalar_mul(onr[:], oT[:, base: base + DH], rec[:])
            out.append(onr)
        return out

    def emit_transpose(p, i, onr):
        h2, qc = divmod(i, 4)
        tr = trpp.tile([64, 128], dtf, tag="tr")
        nc.tensor.transpose(tr[:], onr[:], ident[:])
        nc.vector.tensor_copy(oall[p][h2 * DH:(h2 + 1) * DH, ts(qc, 128)],
                              tr[:])

    prev_norm = None
    for p in range(HP):
        k_items = []
        if p + 2 < HP:
            alloc_kt(p + 2)
            k_items = [(c, e) for c in range(NCH) for e in range(ET)]
        kt_cur = kt_tiles[p]
        k_ps = None
        oT = opsp.tile([128, 1024], dtf, tag="oT")
        ptq = {}
        for j in range(NT):
            dp = dpsp.tile([128, 2 * T], dtf, tag="dp")
            mm(dp[:, 0:T], kt_cur[0:64, ts(j, 128)], qt[p][0:64, :],
               start=True, stop=True)
            mm(dp[:, T:2 * T], kt_cur[64:128, ts(j, 128)], qt[p][64:128, :],
               start=True, stop=True)
            pt = ptp.tile([128, 2 * T], dtb, tag="pt")
            nc.scalar.activation(pt[:], dp[:], AF.Exp)
            ptq[j] = pt
            if prev_norm is not None and j < 8:
                emit_transpose(p - 1, j, prev_norm[j])
            if j >= LAG:
                emit_av(oT, p, j - LAG, ptq.pop(j - LAG))
            # interleave 2 K-chain matmuls for pair p+2
            for _ in range(2):
                if not k_items:
                    continue
                c, e = k_items.pop(0)
                if e == 0:
                    k_ps = kpsp.tile([128, 512], dtf, tag="kps")
                mm(k_ps[:], wk_sb[e][:, ts(p + 2, 128)],
                   hT[e][:, ts(c, 512)],
                   start=(e == 0), stop=(e == ET - 1))
                if e == ET - 1:
                    nc.vector.tensor_scalar_add(
                        kt_tiles[p + 2][:, ts(c, 512)], k_ps[:],
                        bk_s[:, p + 2:p + 3])
        for j in range(NT - LAG, NT):
            emit_av(oT, p, j, ptq.pop(j))
        prev_norm = emit_norm(oT, p)
    for i in range(8):
        emit_transpose(HP - 1, i, prev_norm[i])

    close(trp_cm, kps_cm, ops_cm, dps_cm, rec_cm, onr_cm, pt_cm)
    close(kt_cm, wk_cm, qt_cm, v_cm, hT_cm)
    if stage_limit == "C":
        close(wo_cm, xo_cm, oall_cm, h2_cm, x2_cm, cpool_cm)
        return

    # w1 pool opened early; its DMA overlaps stage D. (w2/g open at stage E.)
    w1_cm, w1p = pool("w1", ET)
    w1_sb = []
    for e in range(ET):
        tw1 = w1p.tile([128, FF], dtb, tag="w1")
        nc.sync.dma_start(out=tw1[:], in_=w1_d[ts(e, 128), :])
        w1_sb.append(tw1)

    # ============ stage D: wo proj + residual + LN2 ============
    x2b_cm, x2bp = pool("x2b", ET)
    sqd_cm, sqdp = pool("sqd", ET)
    abd_cm, abdp = pool("abd", 1)
    prs_cm, prsp = pool("prs", 2, space="PSUM")
    s2s_cm, s2sp = pool("s2s", 1, space="PSUM")
    sq2s_cm, sq2sp = pool("sq2s", 1, space="PSUM")

    x2, x2b = [], []
    S2_ps = s2sp.tile([128, T], dtf, tag="S2")
    for e in range(ET):
        pr_ps = prsp.tile([128, T], dtf, tag="prs")
        for c in range(ET):
            mm(pr_ps[:], wo_sb[c][:, ts(e, 128)], oall[c][:],
               start=(c == 0), stop=(c == ET - 1))
        tx2 = x2p.tile([128, T], dtf, tag="x2")
        nc.vector.tensor_add(tx2[:], pr_ps[:], xo_sb[e][:])
        x2.append(tx2)
        tb = x2bp.tile([128, T], dtb, tag="x2b")
        nc.scalar.activation(tb[:], tx2[:], AF.Copy)
        x2b.append(tb)
        mm(S2_ps[:], ones128[:], tb[:], start=(e == 0), stop=(e == ET - 1))

    h2 = [h2p.tile([128, T], dtb, tag="h2", name=f"h2_{e}")
          for e in range(ET)]
    sq2 = squares(sqdp, [t[:, :] for t in x2b], T)
    SQ2r = pe_sum(sq2sp, "SQ2", [t[:] for t in sq2], T)
    ln_normalize(abdp, [t[:, :] for t in x2b], S2_ps, SQ2r,
                 [t[:, :] for t in h2], T)
    close(sq2s_cm, s2s_cm, prs_cm, abd_cm, sqd_cm, x2b_cm)
    close(wo_cm, xo_cm, oall_cm)
    if stage_limit == "D":
        close(w1_cm, h2_cm, x2_cm, cpool_cm)
        return

    # ============ stage E: FFN ============
    g_cm, gp = pool("g", FT)
    w2_cm, w2p = pool("w2", FT)
    w2_sb = []
    for f in range(FT):
        tw2 = w2p.tile([128, D], dtb, tag="w2", name=f"w2_{f}")
        nc.sync.dma_start(out=tw2[:], in_=w2_d[ts(f, 128), :])
        w2_sb.append(tw2)
    aps_cm, apsp = pool("aps", 3, space="PSUM")
    g_sb = []
    for f in range(FT):
        a_ps = apsp.tile([128, T], dtf, tag="aps")
        for e in range(ET):
            mm(a_ps[:], w1_sb[e][:, ts(f, 128)], h2[e][:],
               start=(e == 0), stop=(e == ET - 1))
        tg = gp.tile([128, T], dtb, tag="g")
        nc.scalar.activation(tg[:], a_ps[:], AF.Gelu, bias=b1e_s[:, f:f + 1])
        g_sb.append(tg)
    close(aps_cm)

    ob_cm, obp = pool("ob", 3)
    yps_cm, ypsp = pool("yps", 2, space="PSUM")
    for e in range(ET):
        y_ps = ypsp.tile([128, T], dtf, tag="yps")
        for f in range(FT):
            mm(y_ps[:], w2_sb[f][:, ts(e, 128)], g_sb[f][:],
               start=(f == 0), stop=(f == FT - 1))
        ob = obp.tile([128, T], dtf, tag="ob")
        nc.vector.tensor_add(ob[:], y_ps[:], x2[e][:])
        nc.vector.tensor_scalar_add(ob[:], ob[:], b2_s[:, e:e + 1])
        nc.sync.dma_start(out=outT_d[ts(e, 128), :], in_=ob[:])
    close(yps_cm, ob_cm, w2_cm, g_cm, w1_cm)

    close(h2_cm, x2_cm, cpool_cm)


_NC_CACHE = {}


def _get_nc():
    if "nc" not in _NC_CACHE:
        _NC_CACHE["nc"] = build()
    return _NC_CACHE["nc"]


def _vec_tiles(v, ntiles):
    return np.ascontiguousarray(
        np.asarray(v, np.float32).reshape(ntiles, 128).T)


def prepare_in_maps(x, wq, wk, wv, wo, w1, b1, w2, b2,
                    ln1_g, ln1_b, ln2_g, ln2_b):
    bf = ml_dtypes.bfloat16
    f32 = np.float32
    x = np.asarray(x, f32)
    wq = np.asarray(wq, f32); wk = np.asarray(wk, f32)
    wv = np.asarray(wv, f32); w1 = np.asarray(w1, f32)
    g1 = np.asarray(ln1_g, f32)[:, None]
    b1v = np.asarray(ln1_b, f32)
    g2 = np.asarray(ln2_g, f32)[:, None]
    b2v = np.asarray(ln2_b, f32)
    bq = (b1v @ wq).astype(f32)          # [D] per-output-col constants
    bk = (b1v @ wk).astype(f32)
    bv = (b1v @ wv).astype(f32)
    b1eff = (np.asarray(b1, f32) + b2v @ w1).astype(f32)
    shared = {
        "wq": np.ascontiguousarray((wq * g1).astype(bf)),
        "wk": np.ascontiguousarray((wk * g1).astype(bf)),
        "wv": np.ascontiguousarray((wv * g1).astype(bf)),
        "wo": np.ascontiguousarray(np.asarray(wo).astype(bf)),
        "w1": np.ascontiguousarray((w1 * g2).astype(bf)),
        "w2": np.ascontiguousarray(np.asarray(w2).astype(bf)),
        "bq": np.ascontiguousarray(bq.reshape(HP, 128).T),
        "bk": np.ascontiguousarray(bk.reshape(HP, 128).T),
        "bvw": np.ascontiguousarray(np.tile(bv.astype(bf), (128, 1))),
        "b1e": _vec_tiles(b1eff, FT),
        "b2": _vec_tiles(b2, ET),
        "ident": np.ascontiguousarray(np.eye(128, dtype=f32)),
    }
    in_maps = []
    for c in range(NCORES):
        b, s = divmod(c, CPB)
        rot = np.concatenate([x[b, s * T:], x[b, :s * T]], axis=0)  # own first
        m = dict(shared)
        m["xbT"] = np.ascontiguousarray(rot.T.astype(bf))
        m["xoT"] = np.ascontiguousarray(x[b, s * T:(s + 1) * T].T)
        in_maps.append(m)
    return in_maps


def assemble_output(results):
    out = np.empty((B, N, D), np.float32)
    for c in range(NCORES):
        b, s = divmod(c, CPB)
        out[b, s * T:(s + 1) * T, :] = results[c]["outT"].T
    return out


def kernel(x, wq, wk, wv, wo, w1, b1, w2, b2, ln1_g, ln1_b, ln2_g, ln2_b):
    from concourse.bass_utils import run_bass_kernel_spmd

    nc = _get_nc()
    in_maps = prepare_in_maps(x, wq, wk, wv, wo, w1, b1, w2, b2,
                              ln1_g, ln1_b, ln2_g, ln2_b)
    res = run_bass_kernel_spmd(nc, in_maps, core_ids=list(range(NCORES)))
    return assemble_output(res.results)


# revision 35
# speedup vs baseline: 1.2667x; 1.0024x over previous
"""Trainium2 Bass kernel for a pre-norm transformer encoder layer.

Problem: x[2,2048,1024]; LN1 -> QKV (16 heads x 64) -> softmax(QK^T) V
-> wo -> +res -> LN2 -> GELU(h@w1+b1)@w2+b2 -> +res.

Sharding: token-parallel over B*N = 4096 tokens; each of the 8 cores owns
512 tokens (cores 0-3: batch 0, cores 4-7: batch 1). Each core recomputes
K/V for its whole batch (no collectives). All activations are kept in
transposed layout [feature, token] so every matmul contracts over the
partition dim. Host pre-rotates each core's batch so its own 512 tokens
are always columns 0:512 -> one NEFF shared by all 8 cores.

v2 restructure (PE-minimizing, cost-model-aware):
 - LN gammas are folded into wq/wk/wv/w1 rows on the host; LN betas become
   per-output-column constants (b@W) applied for free in the PSUM->SBUF
   copies. Device LN is the pure normalize (x-mu)*rstd = x*A - B.
 - LN statistics: sum(x) via a single all-ones [128,128] stationary matmul
   chain (output replicated across partitions, no broadcast matmuls);
   sum(x^2) via a GpSimd (Pool) add tree + partition_all_reduce for the
   steady-state chunks, and via a second PE ones-chain for the
   latency-critical first chunk and for LN2.
 - The LN1 loop is chunk-pipelined (512 tokens at a time): stats/normalize
   of chunk c+1 overlap Q/V/K matmuls of chunk c.
 - Attention AV uses pt (exp dots, keys on partitions) as the *stationary*
   operand so the output is [128 queries, 65] -- full partition use, free
   size 65 instead of 512 (2x less PE time); softmax normalize becomes a
   per-partition tensor_scalar; a PE transpose restores [dh, token] layout.
 - The attention j-loop is software-pipelined: dots/exp lead, AV lags LAG
   slots (so the single-buffered oT PSUM drains across pair boundaries),
   K for pair p+2 is interleaved 2 matmuls per slot to fill the PE gaps
   left by the (ACT-bound) exp, and pair p-1's output transposes ride the
   first 8 slots.
 - FFN2 runs e-outer so output tiles drain early (short tail).

Matmuls run in bf16 with fp32 PSUM accumulation.
"""
import sys
sys.path.insert(0, "/opt/trn_rl_repo")

import numpy as np
import ml_dtypes

import concourse.bass as bass
import concourse.bass_isa as bass_isa
import concourse.tile as tile
from concourse import bacc, mybir

B, N, D = 2, 2048, 1024
H, DH = 16, 64
FF = 4096
NCORES = 8
T = N * B // NCORES          # 512 tokens per core
CPB = NCORES // B            # 4 cores per batch
ET = D // 128                # 8 embed tiles
FT = FF // 128               # 32 ffn tiles
NT = N // 128                # 16 key tiles per batch
NCH = N // 512               # 4 512-chunks per batch
HP = H // 2                  # 8 head pairs

dtb = mybir.dt.bfloat16
dtf = mybir.dt.float32
AF = mybir.ActivationFunctionType
RED = bass_isa.ReduceOp
ts = bass.ts


def build(stage_limit="E"):
    nc = bacc.Bacc("TRN2", target_bir_lowering=False, debug=False)

    xbT_d = nc.dram_tensor("xbT", [D, N], dtb, kind="ExternalInput").ap()
    xoT_d = nc.dram_tensor("xoT", [D, T], dtf, kind="ExternalInput").ap()
    wq_d = nc.dram_tensor("wq", [D, D], dtb, kind="ExternalInput").ap()
    wk_d = nc.dram_tensor("wk", [D, D], dtb, kind="ExternalInput").ap()
    wv_d = nc.dram_tensor("wv", [D, D], dtb, kind="ExternalInput").ap()
    wo_d = nc.dram_tensor("wo", [D, D], dtb, kind="ExternalInput").ap()
    w1_d = nc.dram_tensor("w1", [D, FF], dtb, kind="ExternalInput").ap()
    w2_d = nc.dram_tensor("w2", [FF, D], dtb, kind="ExternalInput").ap()
    bq_d = nc.dram_tensor("bq", [128, HP], dtf, kind="ExternalInput").ap()
    bk_d = nc.dram_tensor("bk", [128, HP], dtf, kind="ExternalInput").ap()
    bvw_d = nc.dram_tensor("bvw", [128, D], dtb, kind="ExternalInput").ap()
    b1e_d = nc.dram_tensor("b1e", [128, FT], dtf, kind="ExternalInput").ap()
    b2_d = nc.dram_tensor("b2", [128, ET], dtf, kind="ExternalInput").ap()
    id_d = nc.dram_tensor("ident", [128, 128], dtf, kind="ExternalInput").ap()
    outT_d = nc.dram_tensor("outT", [D, T], dtf, kind="ExternalOutput").ap()

    with tile.TileContext(nc) as tc:
        _body(nc, tc, xbT_d, xoT_d, wq_d, wk_d, wv_d, wo_d, w1_d, w2_d,
              bq_d, bk_d, bvw_d, b1e_d, b2_d, id_d, outT_d, stage_limit)
    nc.finalize()
    return nc


def _body(nc, tc, xbT_d, xoT_d, wq_d, wk_d, wv_d, wo_d, w1_d, w2_d,
          bq_d, bk_d, bvw_d, b1e_d, b2_d, id_d, outT_d, stage_limit):
    mm = nc.tensor.matmul

    def pool(name, bufs, space="SBUF", side=None):
        cm = tc.tile_pool(name=name, bufs=bufs, space=space, side=side)
        return cm, cm.__enter__()

    def close(*cms):
        for cm in cms:
            cm.__exit__(None, None, None)

    # ---------- persistent pools (right stack) ----------
    cpool_cm, cpool = pool("const", 1, side="right")
    x2_cm, x2p = pool("x2", ET, side="right")
    h2_cm, h2p = pool("h2", ET, side="right")
    oall_cm, oallp = pool("oall", HP, side="right")

    ones128 = cpool.tile([128, 128], dtb)
    nc.vector.memset(ones128[:], 1.0)
    eps128 = cpool.tile([128, 1], dtf)
    nc.vector.memset(eps128[:], 1e-5)
    ident = cpool.tile([128, 128], dtf)
    bq_s = cpool.tile([128, HP], dtf)
    bk_s = cpool.tile([128, HP], dtf)
    bvw_s = cpool.tile([128, D], dtb)
    b1e_s = cpool.tile([128, FT], dtf)
    b2_s = cpool.tile([128, ET], dtf)
    for t_, d_ in ((ident, id_d), (bq_s, bq_d), (bk_s, bk_d),
                   (bvw_s, bvw_d), (b1e_s, b1e_d), (b2_s, b2_d)):
        nc.sync.dma_start(out=t_[:], in_=d_[:, :])

    # ---------- left stack: pools living into the attention phase ----------
    hT_cm, hTp = pool("hT", ET)
    v_cm, vp = pool("v", NT)
    qt_cm, qtp = pool("qt", HP)
    wk_cm, wkp = pool("wk", ET)
    kt_cm, ktp = pool("kt", 3)

    # LN-phase pools
    wq_cm, wqp = pool("wq", ET)
    wv_cm, wvp = pool("wv", ET)
    xb_cm, xbp = pool("xb", 2 * ET)
    sq_cm, sqp = pool("sq", ET + 2)
    tr_cm, trp = pool("tr", 1)
    ab_cm, abp = pool("ab", 1)
    sps_cm, spsp = pool("sps", 2, space="PSUM")
    sqps_cm, sqpsp = pool("sqps", 1, space="PSUM")
    qps_cm, qpsp = pool("qps", 2, space="PSUM")
    vps_cm, vpsp = pool("vps", 2, space="PSUM")
    kps0_cm, kps0p = pool("kps0", 1, space="PSUM")

    # DMA order = emission order (single queue): x chunk 0 first, then wq/wv
    # (needed early), then the rest of x, then wk (needed at attention).
    def load_xchunk(c):
        out = []
        for e in range(ET):
            t = xbp.tile([128, 512], dtb, tag="xb", name=f"xb{c}_{e}")
            nc.sync.dma_start(out=t[:], in_=xbT_d[ts(e, 128), ts(c, 512)])
            out.append(t)
        return out

    xbc = {0: load_xchunk(0)}
    wq_sb, wv_sb = [], []
    for e in range(ET):
        tq = wqp.tile([128, D], dtb, tag="wq")
        nc.sync.dma_start(out=tq[:], in_=wq_d[ts(e, 128), :])
        wq_sb.append(tq)
    for e in range(ET):
        tv = wvp.tile([128, D], dtb, tag="wv")
        nc.sync.dma_start(out=tv[:], in_=wv_d[ts(e, 128), :])
        wv_sb.append(tv)
    for c in range(1, NCH):
        xbc[c] = load_xchunk(c)
    wk_sb = []
    for e in range(ET):
        tk = wkp.tile([128, D], dtb, tag="wk")
        nc.sync.dma_start(out=tk[:], in_=wk_d[ts(e, 128), :])
        wk_sb.append(tk)

    hT = [hTp.tile([128, N], dtb, tag="hT", name=f"hT{e}") for e in range(ET)]
    qt = [qtp.tile([128, T], dtb, tag="qt", name=f"qt{p}") for p in range(HP)]
    v_sb = [vp.tile([128, H * (DH + 1)], dtb, tag="v", name=f"v{j}")
            for j in range(NT)]
    kt_tiles = {}

    def alloc_kt(p):
        kt_tiles[p] = ktp.tile([128, N], dtb, tag="kt", name=f"kt{p}")

    alloc_kt(0)
    alloc_kt(1)

    # ============ stage A+B: per-chunk LN1 -> Q(c0) / V(c) / K0(c) ========
    def ln_normalize(pl, x_slices, S_ps, SQr, out_slices, cw):
        """A = rsqrt(var+eps), B = mean*A; out = x*A - B (all [128, cw])."""
        mean = pl.tile([128, cw], dtf, tag="ab_mean")
        var = pl.tile([128, cw], dtf, tag="ab_var")
        m2 = pl.tile([128, cw], dtf, tag="ab_m2")
        Ar = pl.tile([128, cw], dtf, tag="ab_A")
        Acb = pl.tile([128, cw], dtb, tag="ab_Acb")
        Bcb = pl.tile([128, cw], dtb, tag="ab_Bcb")
        nc.vector.tensor_scalar_mul(mean[:], S_ps[:], 1.0 / D)
        nc.vector.tensor_scalar_mul(var[:], SQr[:], 1.0 / D)
        nc.vector.tensor_mul(m2[:], mean[:], mean[:])
        nc.vector.tensor_sub(var[:], var[:], m2[:])
        nc.scalar.activation(var[:], var[:], AF.Sqrt, bias=eps128[:])
        nc.vector.reciprocal(Ar[:], var[:])
        nc.vector.tensor_mul(m2[:], mean[:], Ar[:])   # B = mean*A
        nc.vector.tensor_copy(Acb[:], Ar[:])
        nc.vector.tensor_copy(Bcb[:], m2[:])
        for xsl, osl in zip(x_slices, out_slices):
            nc.vector.tensor_mul(osl, xsl, Acb[:])
            nc.vector.tensor_sub(osl, osl, Bcb[:])

    def squares(sql, x_slices, cw):
        sq = []
        for xsl in x_slices:
            t = sql.tile([128, cw], dtb, tag="sq")
            nc.vector.tensor_mul(t[:], xsl, xsl)
            sq.append(t)
        return sq

    def pool_sumsq(pl, sq, cw):
        """Serial Pool (GpSimd) adds + partition_all_reduce -> [128, cw]."""
        acc = pl.tile([128, cw], dtf, tag="tr_acc")
        nc.gpsimd.tensor_add(acc[:], sq[0][:], sq[1][:])
        for e in range(2, len(sq)):
            nc.gpsimd.tensor_add(acc[:], acc[:], sq[e][:])
        sqr = pl.tile([128, cw], dtf, tag="tr_r")
        nc.gpsimd.partition_all_reduce(sqr[:], acc[:], 128, RED.add)
        return sqr

    def pe_sum(psp, tag, slices, cw):
        """sum over tiles via all-ones stationary matmul chain (replicated)."""
        s = psp.tile([128, cw], dtf, tag=tag)
        for i, sl in enumerate(slices):
            mm(s[:], ones128[:], sl, start=(i == 0), stop=(i == len(slices) - 1))
        return s

    def k_chain(kt_t, p, c, ps_pool, ps_tag):
        k_ps = ps_pool.tile([128, 512], dtf, tag=ps_tag)
        for e in range(ET):
            mm(k_ps[:], wk_sb[e][:, ts(p, 128)], hT[e][:, ts(c, 512)],
               start=(e == 0), stop=(e == ET - 1))
        nc.vector.tensor_scalar_add(kt_t[:, ts(c, 512)], k_ps[:],
                                    bk_s[:, p:p + 1])

    for c in range(NCH):
        csl = ts(c, 512)
        xc = [xbc[c][e][:, :] for e in range(ET)]
        S_ps = pe_sum(spsp, "S", xc, 512)
        sq = squares(sqp, xc, 512)
        if c == 0:
            # chunk 0 is the critical path: sumsq on the (idle) PE instead of
            # the serial Pool tree to cut first-chunk latency
            SQr = pe_sum(sqpsp, "SQ", [t[:] for t in sq], 512)
        else:
            SQr = pool_sumsq(trp, sq, 512)
        ln_normalize(abp, xc, S_ps, SQr,
                     [hT[e][:, csl] for e in range(ET)], 512)
        # Q (own tokens = chunk 0 only)
        if c == 0:
            for p in range(HP):
                q_ps = qpsp.tile([128, T], dtf, tag="qps")
                for e in range(ET):
                    mm(q_ps[:], wq_sb[e][:, ts(p, 128)], hT[e][:, 0:T],
                       start=(e == 0), stop=(e == ET - 1))
                nc.vector.tensor_scalar_add(qt[p][:], q_ps[:], bq_s[:, p:p + 1])
        # V for this chunk's 4 key tiles
        for j in range(4 * c, 4 * c + 4):
            vt = v_sb[j]
            v3 = vt[:].rearrange("p (h c) -> p h c", c=DH + 1)
            nc.vector.memset(v3[:, :, DH:DH + 1], 1.0)
            for c2 in range(2):
                v_ps = vpsp.tile([128, 512], dtf, tag="vps")
                for e in range(ET):
                    mm(v_ps[:], hT[e][:, ts(j, 128)], wv_sb[e][:, ts(c2, 512)],
                       start=(e == 0), stop=(e == ET - 1))
                bsl = bvw_s[:, ts(c2, 512)].rearrange("p (h c) -> p h c", c=DH)
                nc.vector.tensor_add(
                    v3[:, c2 * 8:(c2 + 1) * 8, 0:DH],
                    v_ps[:].rearrange("p (h c) -> p h c", c=DH), bsl)
        # K pair 0 for this chunk (pairs >= 1 fill attention-phase PE gaps)
        k_chain(kt_tiles[0], 0, c, kps0p, "kps0")
    for c in range(NCH):
        k_chain(kt_tiles[1], 1, c, kps0p, "kps0")

    close(kps0_cm, vps_cm, qps_cm, sqps_cm, sps_cm, ab_cm, tr_cm, sq_cm,
          xb_cm, wv_cm, wq_cm)
    if stage_limit == "A":
        close(kt_cm, wk_cm, qt_cm, v_cm, hT_cm,
              oall_cm, h2_cm, x2_cm, cpool_cm)
        return

    # wo + residual prefetch (right stack; popped after stage D)
    xo_cm, xop = pool("xo", ET, side="right")
    wo_cm, wop = pool("wo", ET, side="right")
    wo_sb, xo_sb = [], []
    for e in range(ET):
        tw = wop.tile([128, D], dtb, tag="wo")
        nc.sync.dma_start(out=tw[:], in_=wo_d[ts(e, 128), :])
        wo_sb.append(tw)
        tx = xop.tile([128, T], dtf, tag="xo")
        nc.sync.dma_start(out=tx[:], in_=xoT_d[ts(e, 128), :])
        xo_sb.append(tx)

    # ============ stage C: attention, software-pipelined ============
    # Per pair p's j-loop: dots/exp lead, AV lags LAG slots (so pair p-1's
    # oT drains before AV(p,0) needs its PSUM slot), K chains for pair p+2
    # fill PE gaps, and pair p-1's transposes ride the first 8 slots.
    LAG = 6
    pt_cm, ptp = pool("pt", LAG + 4)
    onr_cm, onrp = pool("onr", 12)
    rec_cm, recp = pool("rec", 8)
    dps_cm, dpsp = pool("dps", 2, space="PSUM")
    ops_cm, opsp = pool("ops", 1, space="PSUM")
    kps_cm, kpsp = pool("kps", 1, space="PSUM")
    trp_cm, trpp = pool("trp", 1, space="PSUM")

    oall = [oallp.tile([128, T], dtb, tag="oall", name=f"oall{p}")
            for p in range(HP)]

    def emit_av(oT, p, j, pt):
        for h2 in range(2):
            voff = (2 * p + h2) * (DH + 1)
            # one accumulation group per 2KB zero region (bank): start
            # zeroes the whole bank, so the 4 qc-chains share one group
            for qc in range(4):
                mm(oT[:, h2 * 512 + qc * 65: h2 * 512 + qc * 65 + 65],
                   pt[:, h2 * T + qc * 128: h2 * T + (qc + 1) * 128],
                   v_sb[j][:, voff: voff + DH + 1],
                   start=(j == 0 and qc == 0),
                   stop=(j == NT - 1 and qc == 3))

    def emit_norm(oT, p):
        """reciprocal + scale for the 8 (head, qchunk) outputs of pair p;
        returns the normalized [128, DH] tiles (transposed next pair)."""
        out = []
        for i in range(8):
            h2, qc = divmod(i, 4)
            base = h2 * 512 + qc * 65
            rec = recp.tile([128, 1], dtf, tag="rec")
            nc.vector.reciprocal(rec[:], oT[:, base + DH: base + DH + 1])
            onr = onrp.tile([128, DH], dtf, tag="onr", name=f"onr{p}_{i}")
            nc.vector.tensor_scalar_mul(onr[:], oT[:, base: base + DH], rec[:])
            out.append(onr)
        return out

    def emit_transpose(p, i, onr):
        h2, qc = divmod(i, 4)
        tr = trpp.tile([64, 128], dtf, tag="tr")
        nc.tensor.transpose(tr[:], onr[:], ident[:])
        nc.vector.tensor_copy(oall[p][h2 * DH:(h2 + 1) * DH, ts(qc, 128)],
                              tr[:])

    prev_norm = None
    for p in range(HP):
        k_items = []
        if p + 2 < HP:
            alloc_kt(p + 2)
            k_items = [(c, e) for c in range(NCH) for e in range(ET)]
        kt_cur = kt_tiles[p]
        k_ps = None
        oT = opsp.tile([128, 1024], dtf, tag="oT")
        ptq = {}
        for j in range(NT):
            dp = dpsp.tile([128, 2 * T], dtf, tag="dp")
            mm(dp[:, 0:T], kt_cur[0:64, ts(j, 128)], qt[p][0:64, :],
               start=True, stop=True)
            mm(dp[:, T:2 * T], kt_cur[64:128, ts(j, 128)], qt[p][64:128, :],
               start=True, stop=True)
            pt = ptp.tile([128, 2 * T], dtb, tag="pt")
            nc.scalar.activation(pt[:], dp[:], AF.Exp)
            ptq[j] = pt
            if prev_norm is not None and j < 8:
                emit_transpose(p - 1, j, prev_norm[j])
            if j >= LAG:
                emit_av(oT, p, j - LAG, ptq.pop(j - LAG))
            # interleave 2 K-chain matmuls for pair p+2
            for _ in range(2):
                if not k_items:
                    continue
                c, e = k_items.pop(0)
                if e == 0:
                    k_ps = kpsp.tile([128, 512], dtf, tag="kps")
                mm(k_ps[:], wk_sb[e][:, ts(p + 2, 128)],
                   hT[e][:, ts(c, 512)],
                   start=(e == 0), stop=(e == ET - 1))
                if e == ET - 1:
                    nc.vector.tensor_scalar_add(
                        kt_tiles[p + 2][:, ts(c, 512)], k_ps[:],
                        bk_s[:, p + 2:p + 3])
        for j in range(NT - LAG, NT):
            emit_av(oT, p, j, ptq.pop(j))
        prev_norm = emit_norm(oT, p)
    for i in range(8):
        emit_transpose(HP - 1, i, prev_norm[i])

    close(trp_cm, kps_cm, ops_cm, dps_cm, rec_cm, onr_cm, pt_cm)
    close(kt_cm, wk_cm, qt_cm, v_cm, hT_cm)
    if stage_limit == "C":
        close(wo_cm, xo_cm, oall_cm, h2_cm, x2_cm, cpool_cm)
        return

    # w1 pool opened early; its DMA overlaps stage D. (w2/g open at stage E.)
    w1_cm, w1p = pool("w1", ET)
    w1_sb = []
    for e in range(ET):
        tw1 = w1p.tile([128, FF], dtb, tag="w1")
        nc.sync.dma_start(out=tw1[:], in_=w1_d[ts(e, 128), :])
        w1_sb.append(tw1)

    # ============ stage D: wo proj + residual + LN2 ============
    x2b_cm, x2bp = pool("x2b", ET)
    sqd_cm, sqdp = pool("sqd", ET)
    abd_cm, abdp = pool("abd", 1)
    prs_cm, prsp = pool("prs", 2, space="PSUM")
    s2s_cm, s2sp = pool("s2s", 1, space="PSUM")
    sq2s_cm, sq2sp = pool("sq2s", 1, space="PSUM")

    x2, x2b = [], []
    S2_ps = s2sp.tile([128, T], dtf, tag="S2")
    for e in range(ET):
        pr_ps = prsp.tile([128, T], dtf, tag="prs")
        for c in range(ET):
            mm(pr_ps[:], wo_sb[c][:, ts(e, 128)], oall[c][:],
               start=(c == 0), stop=(c == ET - 1))
        tx2 = x2p.tile([128, T], dtf, tag="x2")
        nc.vector.tensor_add(tx2[:], pr_ps[:], xo_sb[e][:])
        x2.append(tx2)
        tb = x2bp.tile([128, T], dtb, tag="x2b")
        nc.scalar.activation(tb[:], tx2[:], AF.Copy)
        x2b.append(tb)
        mm(S2_ps[:], ones128[:], tb[:], start=(e == 0), stop=(e == ET - 1))

    h2 = [h2p.tile([128, T], dtb, tag="h2", name=f"h2_{e}")
          for e in range(ET)]
    sq2 = squares(sqdp, [t[:, :] for t in x2b], T)
    SQ2r = pe_sum(sq2sp, "SQ2", [t[:] for t in sq2], T)
    ln_normalize(abdp, [t[:, :] for t in x2b], S2_ps, SQ2r,
                 [t[:, :] for t in h2], T)
    close(sq2s_cm, s2s_cm, prs_cm, abd_cm, sqd_cm, x2b_cm)
    close(wo_cm, xo_cm, oall_cm)
    if stage_limit == "D":
        close(w1_cm, h2_cm, x2_cm, cpool_cm)
        return

    # ============ stage E: FFN ============
    g_cm, gp = pool("g", FT)
    w2_cm, w2p = pool("w2", FT)
    w2_sb = []
    for f in range(FT):
        tw2 = w2p.tile([128, D], dtb, tag="w2", name=f"w2_{f}")
        nc.sync.dma_start(out=tw2[:], in_=w2_d[ts(f, 128), :])
        w2_sb.append(tw2)
    aps_cm, apsp = pool("aps", 3, space="PSUM")
    g_sb = []
    for f in range(FT):
        a_ps = apsp.tile([128, T], dtf, tag="aps")
        for e in range(ET):
            mm(a_ps[:], w1_sb[e][:, ts(f, 128)], h2[e][:],
               start=(e == 0), stop=(e == ET - 1))
        tg = gp.tile([128, T], dtb, tag="g")
        nc.scalar.activation(tg[:], a_ps[:], AF.Gelu, bias=b1e_s[:, f:f + 1])
        g_sb.append(tg)
    close(aps_cm)

    ob_cm, obp = pool("ob", 4)
    yps_cm, ypsp = pool("yps", 3, space="PSUM")
    for e in range(ET):
        y_ps = ypsp.tile([128, T], dtf, tag="yps")
        for f in range(FT):
            mm(y_ps[:], w2_sb[f][:, ts(e, 128)], g_sb[f][:],
               start=(f == 0), stop=(f == FT - 1))
        ob = obp.tile([128, T], dtf, tag="ob")
        nc.vector.tensor_add(ob[:], y_ps[:], x2[e][:])
        nc.vector.tensor_scalar_add(ob[:], ob[:], b2_s[:, e:e + 1])
        nc.sync.dma_start(out=outT_d[ts(e, 128), :], in_=ob[:])
    close(yps_cm, ob_cm, w2_cm, g_cm, w1_cm)

    close(h2_cm, x2_cm, cpool_cm)


_NC_CACHE = {}


def _get_nc():
    if "nc" not in _NC_CACHE:
        _NC_CACHE["nc"] = build()
    return _NC_CACHE["nc"]


def _vec_tiles(v, ntiles):
    return np.ascontiguousarray(
        np.asarray(v, np.float32).reshape(ntiles, 128).T)


def prepare_in_maps(x, wq, wk, wv, wo, w1, b1, w2, b2,
                    ln1_g, ln1_b, ln2_g, ln2_b):
    bf = ml_dtypes.bfloat16
    f32 = np.float32
    x = np.asarray(x, f32)
    wq = np.asarray(wq, f32); wk = np.asarray(wk, f32)
    wv = np.asarray(wv, f32); w1 = np.asarray(w1, f32)
    g1 = np.asarray(ln1_g, f32)[:, None]
    b1v = np.asarray(ln1_b, f32)
    g2 = np.asarray(ln2_g, f32)[:, None]
    b2v = np.asarray(ln2_b, f32)
    bq = (b1v @ wq).astype(f32)          # [D] per-output-col constants
    bk = (b1v @ wk).astype(f32)
    bv = (b1v @ wv).astype(f32)
    b1eff = (np.asarray(b1, f32) + b2v @ w1).astype(f32)
    shared = {
        "wq": np.ascontiguousarray((wq * g1).astype(bf)),
        "wk": np.ascontiguousarray((wk * g1).astype(bf)),
        "wv": np.ascontiguousarray((wv * g1).astype(bf)),
        "wo": np.ascontiguousarray(np.asarray(wo).astype(bf)),
        "w1": np.ascontiguousarray((w1 * g2).astype(bf)),
        "w2": np.ascontiguousarray(np.asarray(w2).astype(bf)),
        "bq": np.ascontiguousarray(bq.reshape(HP, 128).T),
        "bk": np.ascontiguousarray(bk.reshape(HP, 128).T),
        "bvw": np.ascontiguousarray(np.tile(bv.astype(bf), (128, 1))),
        "b1e": _vec_tiles(b1eff, FT),
        "b2": _vec_tiles(b2, ET),
        "ident": np.ascontiguousarray(np.eye(128, dtype=f32)),
    }
    in_maps = []
    for c in range(NCORES):
        b, s = divmod(c, CPB)
        rot = np.concatenate([x[b, s * T:], x[b, :s * T]], axis=0)  # own first
        m = dict(shared)
        m["xbT"] = np.ascontiguousarray(rot.T.astype(bf))
        m["xoT"] = np.ascontiguousarray(x[b, s * T:(s + 1) * T].T)
        in_maps.append(m)
    return in_maps


def assemble_output(results):
    out = np.empty((B, N, D), np.float32)
    for c in range(NCORES):
        b, s = divmod(c, CPB)
        out[b, s * T:(s + 1) * T, :] = results[c]["outT"].T
    return out


def kernel(x, wq, wk, wv, wo, w1, b1, w2, b2, ln1_g, ln1_b, ln2_g, ln2_b):
    from concourse.bass_utils import run_bass_kernel_spmd

    nc = _get_nc()
    in_maps = prepare_in_maps(x, wq, wk, wv, wo, w1, b1, w2, b2,
                              ln1_g, ln1_b, ln2_g, ln2_b)
    res = run_bass_kernel_spmd(nc, in_maps, core_ids=list(range(NCORES)))
    return assemble_output(res.results)
